# revision 4
# baseline (speedup 1.0000x reference)
"""Channel-transformer (CTR) attention kernel for Trainium2, 8 NeuronCores.

Problem: x (16, 256, 64, 64) f32, gamma scalar.
  xr = x.reshape(B, C, NH, DIM)                       # NH=8, DIM=512
  energy[b,h,c,k] = sum_d xr[b,c,h,d] * xr[b,k,h,d]   # symmetric (C x C)
  attn = softmax(rowmax(energy) - energy, axis=-1)    # == softmax(-energy)
  out[b,c,h,d] = sum_k attn[b,h,c,k] * xr[b,k,h,d]
  result = gamma * out + x
Sharding: data-parallel over batch, 2 samples per core; gamma replicated.

v2 design (per core, all I/O in bf16; host casts f32<->bf16):
  - host ships x twice: c-major XB tiles [128ch, 4096] and d-major XT tiles
    [128d, (h01,kd,c)] packed per head-pair so every DMA row is 4KB.
    (K_XBAR=1 instead builds XT on-device with DMA XBAR transposes.)
  - PE does only the attention math: per (b,h) 8 E matmuls (N=256, bf16)
    and 4 V matmuls (N=512), stream-time floor 27.3us/core.
  - softmax: attnT[kc] = exp(-E - 64) on Scalar with accum_out giving the
    row sums Z (max-shift cancels row-wise; constant bias for range safety;
    E symmetric => exp tile is already k-major for the V matmul).
  - V accumulation order (m0kc0, m1kc0, m0kc1, m1kc1) hides the exp(kc=1)
    latency behind the first two V matmuls.
  - Y accumulated in SBUF bf16, stored per (b, head-pair, m): 2KB rows,
    tail is only the last pair's 512KB.
  - PE warmup matmuls run on a memset tile: no DMA dependency, so the
    clock ramp overlaps the first loads.
"""

import os
import numpy as np

B, C, HW = 16, 256, 4096
NH, DIM = 8, 512
N_CORES = 8
BPC = B // N_CORES  # batches per core
EXP_BIAS = -64.0  # exp(-E + EXP_BIAS): keeps exponents < ~85 for N(0,1) inputs

_CACHE = {}


def _build_module():
    import concourse.bacc as bacc
    import concourse.tile as tile
    import concourse.mybir as mybir

    f32 = mybir.dt.float32
    bf16 = mybir.dt.bfloat16
    AF = mybir.ActivationFunctionType
    OP = mybir.AluOpType

    use_xbar = os.environ.get("K_XBAR", "0") == "1"
    n_warm = int(os.environ.get("K_WARM", "24"))
    _pe = int(os.environ.get("K_PE", "3"))
    _pv = int(os.environ.get("K_PV", "4"))
    _xtb = int(os.environ.get("K_XTB", "3"))

    nc = bacc.Bacc("TRN2", target_bir_lowering=False, debug=False, num_devices=N_CORES)
    xb_d = nc.dram_tensor("xb", [BPC, 2, 128, HW], bf16, kind="ExternalInput").ap()
    if not use_xbar:
        xt_d = nc.dram_tensor("xt", [BPC, 4, 128, 2048], bf16, kind="ExternalInput").ap()
    g_d = nc.dram_tensor("g", [1, 1], f32, kind="ExternalInput").ap()
    y_d = nc.dram_tensor("y", [BPC, 2, 128, HW], bf16, kind="ExternalOutput").ap()

    with tile.TileContext(nc) as tc:
        from contextlib import ExitStack

        with ExitStack() as ctx:
            xb_pool = ctx.enter_context(tc.tile_pool(name="xb", bufs=2 * BPC))
            xt_pool = ctx.enter_context(tc.tile_pool(name="xt", bufs=_xtb))
            y_pool = ctx.enter_context(tc.tile_pool(name="ys", bufs=2 * BPC))
            at_pool = ctx.enter_context(tc.tile_pool(name="at", bufs=4))
            r_pool = ctx.enter_context(tc.tile_pool(name="rp", bufs=12))
            e_pool = ctx.enter_context(tc.tile_pool(name="pe", bufs=_pe, space="PSUM"))
            v_pool = ctx.enter_context(tc.tile_pool(name="pv", bufs=_pv, space="PSUM"))

            cpool = ctx.enter_context(tc.tile_pool(name="const", bufs=1))
            ebias = cpool.tile([128, 1], f32)
            nc.gpsimd.memset(ebias[:], EXP_BIAS)
            onesr = cpool.tile([1, 128], f32)
            nc.gpsimd.memset(onesr[:], 1.0)
            wz = cpool.tile([128, 128], bf16)
            nc.gpsimd.memset(wz[:], 0.0)
            gsb = cpool.tile([1, 1], f32)
            nc.sync.dma_start(gsb[:], g_d[:])
            gamma128 = cpool.tile([128, 1], f32)
            # broadcast gamma to all partitions: [128,1] = ones[1,128].T @ g[1,1]
            gps = v_pool.tile([128, 1], f32, tag="pv", name="gps")
            nc.tensor.matmul(gps[:], onesr[:], gsb[:], start=True, stop=True)
            nc.scalar.copy(gamma128[:], gps[:])

            # PE clock warmup on the zero tile (no DMA dependency)
            warm = e_pool.tile([128, 512], f32, tag="pe", name="warm")
            for _w in range(n_warm):
                nc.tensor.matmul(warm[0:64, 0:64], wz[0:64, 0:64], wz[0:64, 0:64], start=True, stop=True)

            # ---- loads: xt(b,hp) interleaved with xb halves so head 0's
            # operands land first and loads stay just ahead of the PE ----
            XB = [[xb_pool.tile([128, HW], bf16, tag="xb", name=f"XB{b}_{m}") for m in range(2)] for b in range(BPC)]
            XT2 = {}
            Y = [[y_pool.tile([128, HW], bf16, tag="ys", name=f"Y{b}_{m}") for m in range(2)] for b in range(BPC)]

            def load_xt(b, hp):
                t = xt_pool.tile([128, 2, 4, 256], bf16, tag="xt", name=f"XT{b}_{hp}")
                XT2[(b, hp)] = t
                if use_xbar:
                    for h01 in range(2):
                        col = DIM * (2 * hp + h01)
                        nc.sync.dma_start(
                            t[:, h01],
                            xb_d[b, :, :, col : col + DIM],
                            transpose=True,
                        )
                else:
                    nc.sync.dma_start(t[:], xt_d[b, hp])
                return t

            for b in range(BPC):
                load_xt(b, 0)
                load_xt(b, 1)
                for m in range(2):
                    nc.sync.dma_start(XB[b][m][:, 0:2048], xb_d[b, m, :, 0:2048])
                load_xt(b, 2)
                load_xt(b, 3)
                for m in range(2):
                    nc.sync.dma_start(XB[b][m][:, 2048:4096], xb_d[b, m, :, 2048:4096])

            for b in range(BPC):
                for h in range(NH):
                    hp, h01 = h // 2, h % 2
                    col = DIM * h
                    XT = XT2[(b, hp)]

                    # ---- E[:, 256m + k] = energy[c=128m+p, k], accumulated
                    # over the 4 d-chunks; lhsT/rhs both from the d-major tile ----
                    E = e_pool.tile([128, 512], f32, tag="pe", name=f"E{b}_{h}")
                    for m in range(2):
                        for kd in range(4):
                            nc.tensor.matmul(
                                E[:, 256 * m : 256 * (m + 1)],
                                XT[:, h01, kd, 128 * m : 128 * (m + 1)],
                                XT[:, h01, kd],
                                start=(kd == 0),
                                stop=(kd == 3),
                            )

                    # ---- attnT[kc] = exp(-E - 64) (bf16); accum_out = Z ----
                    AT = []
                    Zp = r_pool.tile([128, 2], f32, tag="rp", name=f"Zp{b}_{h}")
                    for kc in range(2):
                        a = at_pool.tile([128, 256], bf16, tag="at", name=f"AT{b}_{h}_{kc}")
                        nc.scalar.activation(
                            a[:], E[:, 256 * kc : 256 * (kc + 1)], AF.Exp,
                            scale=-1.0, bias=ebias[:], accum_out=Zp[:, kc : kc + 1],
                        )
                        AT.append(a)

                    # ---- V[m] += attnT[kc][:, m-half].T @ XB[kc][:, head];
                    # kc-major order so exp(kc=1) hides behind the first two ----
                    V = [v_pool.tile([128, DIM], f32, tag="pv", name=f"V{b}_{h}_{m}") for m in range(2)]
                    for kc in range(2):
                        for m in range(2):
                            nc.tensor.matmul(
                                V[m][:],
                                AT[kc][:, 128 * m : 128 * (m + 1)],
                                XB[b][kc][:, col : col + DIM],
                                start=(kc == 0),
                                stop=(kc == 1),
                            )

                    # ---- Y[m][:, head] = V * (gamma / Z) + XB[m][:, head] ----
                    Rp = r_pool.tile([128, 2], f32, tag="rp", name=f"Rp{b}_{h}")
                    nc.vector.reciprocal(Rp[:], Zp[:])
                    gRp = r_pool.tile([128, 2], f32, tag="rp", name=f"gRp{b}_{h}")
                    nc.gpsimd.tensor_scalar(gRp[:], Rp[:], gamma128[:], None, op0=OP.mult)
                    for m in range(2):
                        nc.vector.scalar_tensor_tensor(
                            Y[b][m][:, col : col + DIM],
                            V[m][:],
                            gRp[:, m : m + 1],
                            XB[b][m][:, col : col + DIM],
                            op0=OP.mult,
                            op1=OP.add,
                        )

                    # ---- store per (b, m, col-half): 4KB rows for DMA rate ----
                    if h == 3 or h == 7:
                        c0 = 2048 * (h // 4)
                        for m in range(2):
                            nc.sync.dma_start(
                                y_d[b, m, :, c0 : c0 + 2048],
                                Y[b][m][:, c0 : c0 + 2048],
                            )

    nc.compile()
    return nc


def _get_module():
    if "nc" not in _CACHE:
        _CACHE["nc"] = _build_module()
    return _CACHE["nc"]


def _make_in_maps(x_np, g_np):
    """Shard + pack FULL inputs into the per-core DRAM tensors (bf16)."""
    import ml_dtypes

    bf16 = ml_dtypes.bfloat16
    x = np.ascontiguousarray(np.asarray(x_np, dtype=np.float32)).reshape(B, C, HW)
    xbf = x.astype(bf16)
    xb = np.ascontiguousarray(xbf.reshape(B, 2, 128, HW))
    g = np.asarray(g_np, dtype=np.float32).reshape(1, 1)

    use_xbar = os.environ.get("K_XBAR", "0") == "1"
    if use_xbar:
        xt = None
    else:
        # xt[b, hp, p, h01, kd, c] = x[b, c, 512*(2hp+h01) + 128kd + p]
        xtv = xbf.reshape(B, C, NH // 2, 2, 4, 128)
        xt = np.ascontiguousarray(xtv.transpose(0, 2, 5, 3, 4, 1)).reshape(B, NH // 2, 128, 2048)

    maps = []
    for i in range(N_CORES):
        m = {
            "xb": np.ascontiguousarray(xb[i * BPC : (i + 1) * BPC]),
            "g": g,
        }
        if xt is not None:
            m["xt"] = np.ascontiguousarray(xt[i * BPC : (i + 1) * BPC])
        maps.append(m)
    return maps


def kernel(x_input, gamma):
    from concourse.bass_utils import run_bass_kernel_spmd

    nc = _get_module()
    in_maps = _make_in_maps(x_input, gamma)
    res = run_bass_kernel_spmd(nc, in_maps, list(range(N_CORES)))
    y = np.concatenate([np.asarray(res.results[i]["y"]) for i in range(N_CORES)], axis=0)
    # y is [B, 2, 128, HW] bf16 with channels = 128*m + p
    return y.reshape(B, C, 64, 64).astype(np.float32)


# revision 7
# speedup vs baseline: 1.0208x; 1.0208x over previous
"""Channel-transformer (CTR) attention kernel for Trainium2, 8 NeuronCores.

Problem: x (16, 256, 64, 64) f32, gamma scalar.
  xr = x.reshape(B, C, NH, DIM)                       # NH=8, DIM=512
  energy[b,h,c,k] = sum_d xr[b,c,h,d] * xr[b,k,h,d]   # symmetric (C x C)
  attn = softmax(rowmax(energy) - energy, axis=-1)    # == softmax(-energy)
  out[b,c,h,d] = sum_k attn[b,h,c,k] * xr[b,k,h,d]
  result = gamma * out + x
Sharding: data-parallel over batch, 2 samples per core; gamma replicated.

v2 design (per core, all I/O in bf16; host casts f32<->bf16):
  - host ships x twice: c-major XB tiles [128ch, 4096] and d-major XT tiles
    [128d, (h01,kd,c)] packed per head-pair so every DMA row is 4KB.
    (K_XBAR=1 instead builds XT on-device with DMA XBAR transposes.)
  - PE does only the attention math: per (b,h) 8 E matmuls (N=256, bf16)
    and 4 V matmuls (N=512), stream-time floor 27.3us/core.
  - softmax: attnT[kc] = exp(-E - 64) on Scalar with accum_out giving the
    row sums Z (max-shift cancels row-wise; constant bias for range safety;
    E symmetric => exp tile is already k-major for the V matmul).
  - V accumulation order (m0kc0, m1kc0, m0kc1, m1kc1) hides the exp(kc=1)
    latency behind the first two V matmuls.
  - Y accumulated in SBUF bf16, stored per (b, head-pair, m): 2KB rows,
    tail is only the last pair's 512KB.
  - PE warmup matmuls run on a memset tile: no DMA dependency, so the
    clock ramp overlaps the first loads.
"""

import os
import numpy as np

B, C, HW = 16, 256, 4096
NH, DIM = 8, 512
N_CORES = 8
BPC = B // N_CORES  # batches per core
EXP_BIAS = -64.0  # exp(-E + EXP_BIAS): keeps exponents < ~85 for N(0,1) inputs

_CACHE = {}


def _build_module():
    import concourse.bacc as bacc
    import concourse.tile as tile
    import concourse.mybir as mybir

    f32 = mybir.dt.float32
    bf16 = mybir.dt.bfloat16
    AF = mybir.ActivationFunctionType
    OP = mybir.AluOpType

    use_xbar = os.environ.get("K_XBAR", "0") == "1"
    n_warm = int(os.environ.get("K_WARM", "52"))
    _pe = int(os.environ.get("K_PE", "3"))
    _pv = int(os.environ.get("K_PV", "4"))
    _xtb = int(os.environ.get("K_XTB", "4"))
    split_stt = os.environ.get("K_SPLIT", "0") == "1"  # GpSimd cannot read PSUM

    nc = bacc.Bacc("TRN2", target_bir_lowering=False, debug=False, num_devices=N_CORES)
    xb_d = nc.dram_tensor("xb", [BPC, 2, 128, HW], bf16, kind="ExternalInput").ap()
    if not use_xbar:
        xt_d = nc.dram_tensor("xt", [BPC, 4, 128, 2048], bf16, kind="ExternalInput").ap()
    g_d = nc.dram_tensor("g", [1, 1], f32, kind="ExternalInput").ap()
    y_d = nc.dram_tensor("y", [BPC, 2, 128, HW], bf16, kind="ExternalOutput").ap()

    with tile.TileContext(nc) as tc:
        from contextlib import ExitStack

        with ExitStack() as ctx:
            xb_pool = ctx.enter_context(tc.tile_pool(name="xb", bufs=2 * BPC))
            xt_pool = ctx.enter_context(tc.tile_pool(name="xt", bufs=_xtb))
            y_pool = ctx.enter_context(tc.tile_pool(name="ys", bufs=2 * BPC))
            at_pool = ctx.enter_context(tc.tile_pool(name="at", bufs=4))
            r_pool = ctx.enter_context(tc.tile_pool(name="rp", bufs=12))
            e_pool = ctx.enter_context(tc.tile_pool(name="pe", bufs=_pe, space="PSUM"))
            v_pool = ctx.enter_context(tc.tile_pool(name="pv", bufs=_pv, space="PSUM"))

            cpool = ctx.enter_context(tc.tile_pool(name="const", bufs=1))
            ebias = cpool.tile([128, 1], f32)
            nc.gpsimd.memset(ebias[:], EXP_BIAS)
            onesr = cpool.tile([1, 128], f32)
            nc.gpsimd.memset(onesr[:], 1.0)
            wz = cpool.tile([128, 128], bf16)
            nc.gpsimd.memset(wz[:], 0.0)
            gsb = cpool.tile([1, 1], f32)
            nc.sync.dma_start(gsb[:], g_d[:])
            gamma128 = cpool.tile([128, 1], f32)
            # broadcast gamma to all partitions: [128,1] = ones[1,128].T @ g[1,1]
            gps = v_pool.tile([128, 1], f32, tag="pv", name="gps")
            nc.tensor.matmul(gps[:], onesr[:], gsb[:], start=True, stop=True)
            nc.scalar.copy(gamma128[:], gps[:])

            # PE clock warmup on the zero tile (no DMA dependency)
            warm = e_pool.tile([128, 512], f32, tag="pe", name="warm")
            for _w in range(n_warm):
                nc.tensor.matmul(warm[0:64, 0:64], wz[0:64, 0:64], wz[0:64, 0:64], start=True, stop=True)

            # ---- loads: xt(b,hp) interleaved with xb halves so head 0's
            # operands land first and loads stay just ahead of the PE ----
            XB = [[xb_pool.tile([128, HW], bf16, tag="xb", name=f"XB{b}_{m}") for m in range(2)] for b in range(BPC)]
            XT2 = {}
            Y = [[y_pool.tile([128, HW], bf16, tag="ys", name=f"Y{b}_{m}") for m in range(2)] for b in range(BPC)]

            def load_xt(b, hp):
                t = xt_pool.tile([128, 2, 4, 256], bf16, tag="xt", name=f"XT{b}_{hp}")
                XT2[(b, hp)] = t
                if use_xbar:
                    for h01 in range(2):
                        col = DIM * (2 * hp + h01)
                        nc.sync.dma_start(
                            t[:, h01],
                            xb_d[b, :, :, col : col + DIM],
                            transpose=True,
                        )
                else:
                    nc.sync.dma_start(t[:], xt_d[b, hp])
                return t

            for b in range(BPC):
                load_xt(b, 0)
                load_xt(b, 1)
                for m in range(2):
                    nc.sync.dma_start(XB[b][m][:, 0:2048], xb_d[b, m, :, 0:2048])
                load_xt(b, 2)
                load_xt(b, 3)
                for m in range(2):
                    nc.sync.dma_start(XB[b][m][:, 2048:4096], xb_d[b, m, :, 2048:4096])

            # ---- software-pipelined head loop: PE order is
            #   E(t)m0, E(t)m1, V(t)kc0, E(t+1)m0, V(t)kc1, E(t+1)m1, ...
            # so exp(t,kc1) hides behind V(t)kc0 + E(t+1)m0 and the PE
            # never waits on the Scalar engine ----
            heads = [(b, h) for b in range(BPC) for h in range(NH)]
            T = len(heads)
            st = {}  # t -> dict(E, AT, Zp, V)

            def emit_E_group(t, m):
                b, h = heads[t]
                hp, h01 = h // 2, h % 2
                XT = XT2[(b, hp)]
                if m == 0:
                    st[t] = {
                        "E": e_pool.tile([128, 512], f32, tag="pe", name=f"E{b}_{h}"),
                        "AT": [],
                        "Zp": r_pool.tile([128, 2], f32, tag="rp", name=f"Zp{b}_{h}"),
                    }
                E = st[t]["E"]
                for kd in range(4):
                    nc.tensor.matmul(
                        E[:, 256 * m : 256 * (m + 1)],
                        XT[:, h01, kd, 128 * m : 128 * (m + 1)],
                        XT[:, h01, kd],
                        start=(kd == 0),
                        stop=(kd == 3),
                    )
                # attnT[kc=m] = exp(-E - 64) (bf16); accum_out = row sums Z
                a = at_pool.tile([128, 256], bf16, tag="at", name=f"AT{b}_{h}_{m}")
                nc.scalar.activation(
                    a[:], E[:, 256 * m : 256 * (m + 1)], AF.Exp,
                    scale=-1.0, bias=ebias[:], accum_out=st[t]["Zp"][:, m : m + 1],
                )
                st[t]["AT"].append(a)

            def emit_V_group(t, kc):
                b, h = heads[t]
                col = DIM * h
                if kc == 0:
                    st[t]["V"] = [
                        v_pool.tile([128, DIM], f32, tag="pv", name=f"V{b}_{h}_{m}")
                        for m in range(2)
                    ]
                V, AT = st[t]["V"], st[t]["AT"]
                for m in range(2):
                    nc.tensor.matmul(
                        V[m][:],
                        AT[kc][:, 128 * m : 128 * (m + 1)],
                        XB[b][kc][:, col : col + DIM],
                        start=(kc == 0),
                        stop=(kc == 1),
                    )

            def emit_epilogue(t):
                b, h = heads[t]
                col = DIM * h
                V, Zp = st[t]["V"], st[t]["Zp"]
                # Y[m][:, head] = V * (gamma / Z) + XB[m][:, head]
                Rp = r_pool.tile([128, 2], f32, tag="rp", name=f"Rp{b}_{h}")
                nc.vector.reciprocal(Rp[:], Zp[:])
                gRp = r_pool.tile([128, 2], f32, tag="rp", name=f"gRp{b}_{h}")
                nc.vector.tensor_scalar(gRp[:], Rp[:], gamma128[:], None, op0=OP.mult)
                for m in range(2):
                    eng = nc.gpsimd if (split_stt and m == 1) else nc.vector
                    eng.scalar_tensor_tensor(
                        Y[b][m][:, col : col + DIM],
                        V[m][:],
                        gRp[:, m : m + 1],
                        XB[b][m][:, col : col + DIM],
                        op0=OP.mult,
                        op1=OP.add,
                    )
                # stores: 4KB rows mid-stream, small final chunk for a short tail
                ranges = {3: (0, 2048), 6: (2048, 3584), 7: (3584, 4096)}
                if h in ranges:
                    c0, c1 = ranges[h]
                    for m in range(2):
                        nc.sync.dma_start(
                            y_d[b, m, :, c0:c1], Y[b][m][:, c0:c1]
                        )
                del st[t]

            emit_E_group(0, 0)
            emit_E_group(0, 1)
            for t in range(T):
                emit_V_group(t, 0)
                if t + 1 < T:
                    emit_E_group(t + 1, 0)
                emit_V_group(t, 1)
                if t + 1 < T:
                    emit_E_group(t + 1, 1)
                emit_epilogue(t)

    nc.compile()
    return nc


def _get_module():
    if "nc" not in _CACHE:
        _CACHE["nc"] = _build_module()
    return _CACHE["nc"]


def _make_in_maps(x_np, g_np):
    """Shard + pack FULL inputs into the per-core DRAM tensors (bf16)."""
    import ml_dtypes

    bf16 = ml_dtypes.bfloat16
    x = np.ascontiguousarray(np.asarray(x_np, dtype=np.float32)).reshape(B, C, HW)
    xbf = x.astype(bf16)
    xb = np.ascontiguousarray(xbf.reshape(B, 2, 128, HW))
    g = np.asarray(g_np, dtype=np.float32).reshape(1, 1)

    use_xbar = os.environ.get("K_XBAR", "0") == "1"
    if use_xbar:
        xt = None
    else:
        # xt[b, hp, p, h01, kd, c] = x[b, c, 512*(2hp+h01) + 128kd + p]
        xtv = xbf.reshape(B, C, NH // 2, 2, 4, 128)
        xt = np.ascontiguousarray(xtv.transpose(0, 2, 5, 3, 4, 1)).reshape(B, NH // 2, 128, 2048)

    maps = []
    for i in range(N_CORES):
        m = {
            "xb": np.ascontiguousarray(xb[i * BPC : (i + 1) * BPC]),
            "g": g,
        }
        if xt is not None:
            m["xt"] = np.ascontiguousarray(xt[i * BPC : (i + 1) * BPC])
        maps.append(m)
    return maps


def kernel(x_input, gamma):
    from concourse.bass_utils import run_bass_kernel_spmd

    nc = _get_module()
    in_maps = _make_in_maps(x_input, gamma)
    res = run_bass_kernel_spmd(nc, in_maps, list(range(N_CORES)))
    y = np.concatenate([np.asarray(res.results[i]["y"]) for i in range(N_CORES)], axis=0)
    # y is [B, 2, 128, HW] bf16 with channels = 128*m + p
    return y.reshape(B, C, 64, 64).astype(np.float32)
